# revision 23
# baseline (speedup 1.0000x reference)
"""MultiHeadAttention (N=2, S=T=4096, E=768, H=12, DH=64) on 8 NeuronCores.

Sharding: batch x head-group. Core k handles batch n=k//4 and heads
[3g, 3g+3) with g=k%4. Each core computes Q/K/V projections for its 3
heads, causal attention, and a partial output projection (tensor-parallel
split of Wo along the head dim, bias/4 folded in via a ones row). Host
sums the 4 partials per batch.

Device pipeline (per core):
- Scores are computed transposed (S^T[t, q]) per 128-t block with the
  two tb of a pair side by side in one [128, 1024] PSUM tile, so one
  ACTIVATE exponentiates both (amortizing the ~352-cycle ACT overhead).
- The causal mask is synthesized on device as a [128, 512] bf16
  staircase (affine_select) and multiplied into the diagonal blocks
  after exp; off-diagonal blocks are skipped/unmasked by construction.
- attn@V runs with the exp-scores as the matmul *stationary* operand
  ([t=128, q=128] chunks, full contraction height) and V (+ ones
  column) moving, accumulating Y as [q, 4, 65] per head in one PSUM
  bank; the 65th column is the softmax denominator, already laid out
  per-partition, so normalization is one DVE reciprocal plus four
  [128, 64] tensor_scalar multiplies.
- Normalized heads are transposed back to [d, q] with PE transpose
  matmuls (bf16 PSUM) for the output projection.
- Projections are software-pipelined: proj(b+2)'s matmuls are emitted
  in quanta between attention pairs of query-block b so the scalar
  engine (the attention-phase bottleneck) never starves while the PE
  runs projections.
"""

import os
import sys

for _p in ("/opt/trn_rl_repo",):
    if _p not in sys.path and os.path.isdir(_p):
        sys.path.append(_p)

import numpy as np
import ml_dtypes

import concourse.bass as bass
import concourse.tile as tile
from concourse import mybir
from concourse.bass_utils import run_bass_kernel_spmd
from concourse.masks import make_identity

BF16 = ml_dtypes.bfloat16
F32 = mybir.dt.float32
BF = mybir.dt.bfloat16

N, S, T, E, H = 2, 4096, 4096, 768, 12
DH = 64
HPC = 3            # heads per core
NH3 = HPC * DH     # 192
QB = 512           # query block
TB = 128           # key/t block (partition dim of score tiles)
NQB = S // QB      # 8
NTB = T // TB      # 32
EC = E // 128      # 6 contraction chunks for projections


# ---------------------------------------------------------------------------
# This walrus build rejects instructions carrying more than MAX_WAITS sem
# waits. After Tile scheduling, hoist excess waits onto single-wait nops
# inserted just before the offending instruction on the same engine.
MAX_WAITS = 1


def _split_waits(nc, limit=MAX_WAITS):
    ctr = 0
    for bb in nc.m.functions[0].blocks:
        out = []
        dirty = False
        for inst in bb.instructions:
            si = inst.sync_info
            nw = len(si.on_wait) if (si and si.on_wait) else 0
            if nw > limit:
                waits = list(si.on_wait)
                for w in waits[:-limit]:
                    ctr += 1
                    out.append(
                        mybir.InstNoOp(
                            name=f"bass_waitsplit_{ctr}",
                            engine=inst.engine,
                            sync_info=mybir.SyncInfo(on_wait=[w], on_update=[]),
                            bass_nofuse=True,
                        )
                    )
                inst.sync_info = mybir.SyncInfo(
                    on_wait=waits[-limit:], on_update=list(si.on_update or [])
                )
                dirty = True
            out.append(inst)
        if dirty:
            bb.instructions = out
    return nc


# ---------------------------------------------------------------------------
def _build():
    nc = bass.Bass("TRN2", target_bir_lowering=False, debug=False, num_devices=8)

    qt = nc.declare_dram_parameter("qt", [E, S], BF, isOutput=False)
    kt = nc.declare_dram_parameter("kt", [E, S], BF, isOutput=False)
    vt = nc.declare_dram_parameter("vt", [E, S], BF, isOutput=False)
    wq = nc.declare_dram_parameter("wq", [E, NH3], BF, isOutput=False)
    wk = nc.declare_dram_parameter("wk", [E, NH3], BF, isOutput=False)
    wv = nc.declare_dram_parameter("wv", [E, NH3], BF, isOutput=False)
    bqkv = nc.declare_dram_parameter("bqkv", [3, NH3], F32, isOutput=False)
    wo = nc.declare_dram_parameter("wo", [NH3 + 1, E], BF, isOutput=False)
    out = nc.declare_dram_parameter("out", [S, E], F32, isOutput=True)

    with tile.TileContext(nc) as tc:
        with (
            tc.tile_pool(name="consts", bufs=1) as consts,
            tc.tile_pool(name="persist", bufs=1) as persist,
            tc.tile_pool(name="xq", bufs=3) as xqpool,
            tc.tile_pool(name="xv", bufs=3) as xvpool,
            tc.tile_pool(name="ea", bufs=4) as epool,
            tc.tile_pool(name="ysb", bufs=2) as spool,
            tc.tile_pool(name="yab", bufs=2) as ypool,
            tc.tile_pool(name="recs", bufs=2) as rpool,
            tc.tile_pool(name="osb", bufs=2) as opool,
            tc.tile_pool(name="ps_s", bufs=2, space="PSUM") as ps_s,
            tc.tile_pool(name="ps_y", bufs=1, space="PSUM") as ps_y,
            tc.tile_pool(name="ps_t", bufs=1, space="PSUM") as ps_t,
            tc.tile_pool(name="ps_o", bufs=1, space="PSUM") as ps_o,
        ):
            # ---- constants (gpsimd queue; x DMAs own sync/scalar) -----------
            wq_sb = consts.tile([128, EC, NH3], BF, tag="wq")
            nc.gpsimd.dma_start(out=wq_sb, in_=wq.rearrange("(c p) m -> p c m", p=128))
            wk_sb = consts.tile([128, EC, NH3], BF, tag="wk")
            nc.gpsimd.dma_start(out=wk_sb, in_=wk.rearrange("(c p) m -> p c m", p=128))
            wv_sb = consts.tile([128, EC, NH3], BF, tag="wv")
            nc.gpsimd.dma_start(out=wv_sb, in_=wv.rearrange("(c p) m -> p c m", p=128))
            wo0 = consts.tile([128, E], BF, tag="wo0")
            nc.gpsimd.dma_start(out=wo0, in_=wo[0:128, :])
            wo1 = consts.tile([NH3 + 1 - 128, E], BF, tag="wo1")
            nc.gpsimd.dma_start(out=wo1, in_=wo[128 : NH3 + 1, :])
            # per-partition (d) bias scalars: [64, tensor, head]
            bias_sb = consts.tile([DH, 3, HPC], F32, tag="bias")
            nc.gpsimd.dma_start(
                out=bias_sb, in_=bqkv.rearrange("t (h d) -> d t h", d=DH)
            )
            # V bias broadcast along partitions: [128, HPC, DH]
            bv_sb = consts.tile([128, HPC, DH], F32, tag="bv")
            bsrc = bqkv[2:3, :]
            nc.gpsimd.dma_start(
                out=bv_sb,
                in_=bass.AP(
                    tensor=bsrc.tensor,
                    offset=bsrc.offset,
                    ap=[[0, 128], [DH, HPC], [1, DH]],
                ),
            )
            # causal staircase: stair[t, y] = 1.0 if y >= t else 0.0
            stair = consts.tile([128, QB], BF, tag="stair")
            nc.gpsimd.memset(stair, 1.0)
            nc.gpsimd.affine_select(
                out=stair,
                in_=stair,
                compare_op=mybir.AluOpType.is_ge,
                fill=0.0,
                base=0,
                pattern=[[1, QB]],
                channel_multiplier=-1,
            )
            # identity for PE transposes
            ident = consts.tile([128, 128], BF, tag="ident")
            make_identity(nc, ident)

            # per-block projected activations (dup'd across partition halves
            # so score matmuls can alternate PE row tiles)
            qt_b = [
                persist.tile([128, HPC, QB], BF, tag=f"qtb{b}", name=f"qt_b{b}")
                for b in range(NQB)
            ]
            kt_b = [
                persist.tile([128, HPC, QB], BF, tag=f"ktb{b}", name=f"kt_b{b}")
                for b in range(NQB)
            ]
            v_t = [
                persist.tile([128, HPC, DH + 1], BF, tag=f"v{t}", name=f"v_t{t}")
                for t in range(NTB)
            ]

            # keep big input DMAs OFF the scalar queue: its strict 8-deep
            # FIFO would stall exps behind a waiting dma_start
            dma_q = [nc.sync, nc.sync, nc.gpsimd]
            dq = [0]

            def _dma(out_, in_):
                eng = dma_q[dq[0] % 3]
                dq[0] += 1
                eng.dma_start(out=out_, in_=in_)

            def emit_proj_dmas(b):
                """Issue the three x-block DMAs for projection block b."""
                xk = xqpool.tile([128, EC, QB], BF, tag="xk", name=f"xk{b}")
                _dma(xk, kt[:, b * QB : (b + 1) * QB].rearrange("(c p) s -> p c s", p=128))
                xq = xqpool.tile([128, EC, QB], BF, tag="xq", name=f"xq{b}")
                _dma(xq, qt[:, b * QB : (b + 1) * QB].rearrange("(c p) s -> p c s", p=128))
                xv = xvpool.tile([128, EC, QB], BF, tag="xv", name=f"xv{b}")
                _dma(xv, vt[:, b * QB : (b + 1) * QB].rearrange("(c p) s -> p c s", p=128))
                return xk, xq, xv

            def qk_pieces01(b, xblk, wsb, ti, dst, split):
                """Heads 0/1 projection; optionally split into two 3-MM
                halves that share one ring tile across an intervening pair."""
                st = {}

                def fst():
                    st["ps"] = ps_s.tile(
                        [128, 2 * QB], F32, tag="sps", name=f"p{ti}a{b}"
                    )
                    for c in range(EC // 2):
                        nc.tensor.matmul(
                            st["ps"][:, 0:QB], wsb[:, c, 0:128], xblk[:, c, :],
                            start=(c == 0), stop=False, skip_group_check=True,
                        )

                def snd():
                    ps = st["ps"]
                    for c in range(EC // 2, EC):
                        nc.tensor.matmul(
                            ps[:, 0:QB], wsb[:, c, 0:128], xblk[:, c, :],
                            start=False, stop=(c == EC - 1),
                            skip_group_check=True,
                        )
                    for h in range(2):
                        nc.vector.tensor_scalar_add(
                            dst[0:DH, h, :],
                            ps[h * DH : (h + 1) * DH, 0:QB],
                            bias_sb[:, ti : ti + 1, h],
                        )

                if split:
                    return [fst, snd]
                return [lambda: (fst(), snd())]

            def qk_pieces2(b, xblk, wsb, ti, dst, split):
                """Head 2 projection + duplicate into the upper half."""
                st = {}

                def fst():
                    st["ps"] = ps_s.tile(
                        [128, 2 * QB], F32, tag="sps", name=f"p{ti}b{b}"
                    )
                    for c in range(EC // 2):
                        nc.tensor.matmul(
                            st["ps"][0:DH, 0:QB], wsb[:, c, 128:NH3], xblk[:, c, :],
                            start=(c == 0), stop=False, skip_group_check=True,
                        )

                def snd():
                    ps = st["ps"]
                    for c in range(EC // 2, EC):
                        nc.tensor.matmul(
                            ps[0:DH, 0:QB], wsb[:, c, 128:NH3], xblk[:, c, :],
                            start=False, stop=(c == EC - 1),
                            skip_group_check=True,
                        )
                    nc.vector.tensor_scalar_add(
                        dst[0:DH, 2, :], ps[0:DH, 0:QB], bias_sb[:, ti : ti + 1, 2]
                    )
                    nc.vector.tensor_copy(dst[DH:128, :, :], dst[0:DH, :, :])

                if split:
                    return [fst, snd]
                return [lambda: (fst(), snd())]

            def v_pieces(b, xv, i, split):
                st = {}

                def fst():
                    st["ps"] = ps_s.tile(
                        [128, 2 * QB], F32, tag="sps", name=f"pv{b}_{i}"
                    )
                    for c in range(EC // 2):
                        nc.tensor.matmul(
                            st["ps"][:, 0:NH3],
                            xv[:, c, i * 128 : (i + 1) * 128],
                            wv_sb[:, c, :],
                            start=(c == 0), stop=False, skip_group_check=True,
                        )

                def snd():
                    ps = st["ps"]
                    for c in range(EC // 2, EC):
                        nc.tensor.matmul(
                            ps[:, 0:NH3],
                            xv[:, c, i * 128 : (i + 1) * 128],
                            wv_sb[:, c, :],
                            start=False, stop=(c == EC - 1),
                            skip_group_check=True,
                        )
                    tb = 4 * b + i
                    nc.vector.memset(v_t[tb][:, :, DH : DH + 1], 1.0)
                    psv = ps[:, 0:NH3]
                    nc.vector.tensor_add(
                        v_t[tb][:, :, 0:DH],
                        bass.AP(
                            tensor=psv.tensor,
                            offset=psv.offset,
                            ap=[[psv.ap[0][0], 128], [DH, HPC], [1, DH]],
                        ),
                        bv_sb,
                    )

                if split:
                    return [fst, snd]
                return [lambda: (fst(), snd())]

            def proj_quanta(b, dmas, split=False):
                xk, xq, xv = dmas
                return (
                    qk_pieces01(b, xk, wk_sb, 1, kt_b[b], split)
                    + qk_pieces2(b, xk, wk_sb, 1, kt_b[b], split)
                    + qk_pieces01(b, xq, wq_sb, 0, qt_b[b], split)
                    + qk_pieces2(b, xq, wq_sb, 0, qt_b[b], split)
                    + v_pieces(b, xv, 0, split)
                    + v_pieces(b, xv, 1, split)
                    + v_pieces(b, xv, 2, split)
                    + v_pieces(b, xv, 3, split)
                )

            def output_pieces(b, ysb_b):
                """Transpose + output projection for block b, as small pieces
                that interleave into the next block's pair stream."""
                st = [None]

                def t_piece():
                    yt = ps_t.tile([128, 2 * QB], BF, tag="yt", name=f"yt{b}")
                    for c in range(4):
                        nc.tensor.transpose(
                            yt[:, c * 128 : (c + 1) * 128], ysb_b[:, c, 0:128], ident
                        )
                        nc.tensor.transpose(
                            yt[0:DH, QB + c * 128 : QB + (c + 1) * 128],
                            ysb_b[:, c, 128:NH3],
                            ident,
                        )
                    ya = ypool.tile([128, QB], BF, tag="ya", name=f"ya{b}")
                    yb = ypool.tile([DH + 1, QB], BF, tag="yb", name=f"yb{b}")
                    nc.vector.tensor_copy(ya, yt[:, 0:QB])
                    nc.vector.tensor_copy(yb[0:DH, :], yt[0:DH, QB : 2 * QB])
                    nc.vector.memset(yb[DH : DH + 1, :], 1.0)
                    st[0] = (ya, yb)

                def o_piece(ss):
                    ya, yb = st[0]
                    ssl = slice(ss * 128, (ss + 1) * 128)
                    for eh in (0, 1):
                        esl = slice(eh * 384, (eh + 1) * 384)
                        po = ps_o.tile(
                            [128, 384], F32, tag=f"po{eh}", name=f"po{b}_{ss}_{eh}"
                        )
                        nc.tensor.matmul(
                            po, ya[:, ssl], wo0[:, esl], start=True, stop=False
                        )
                        nc.tensor.matmul(
                            po, yb[:, ssl], wo1[:, esl], start=False, stop=True
                        )
                        osb = opool.tile([128, 384], F32, tag="osb")
                        nc.vector.tensor_copy(osb, po)
                        nc.gpsimd.dma_start(
                            out=out[b * QB + ss * 128 : b * QB + (ss + 1) * 128, esl],
                            in_=osb,
                        )

                return [t_piece] + [
                    (lambda ss=ss: o_piece(ss)) for ss in range(4)
                ]

            # ---- prologue: projections for block 0, except the last two V
            # pieces (not needed until the second/third attn@V flush) --------
            dmas = {0: emit_proj_dmas(0), 1: emit_proj_dmas(1)}
            q0 = proj_quanta(0, dmas[0])
            for q in q0[:6]:
                q()
            pend_prologue = q0[6:]

            sc_par = [0]  # running parity for score row-tiling
            pend_out = []  # deferred output pieces from the previous block
            # One continuous (b, h, pair) stream: the attn@V flush and the
            # per-head normalization for a step are emitted inside the NEXT
            # step, so the PE/ACT pipeline never drains at h/b boundaries.
            pending = [None]  # (ea, halves, b, h, yps, after_cb)

            def _flush():
                if pending[0] is None:
                    return
                ea_p, halves_p, b_p, h_p, yps_p, after = pending[0]
                for half, tb in sorted(halves_p, key=lambda x: x[1]):
                    i = tb - 4 * b_p
                    for c in range(max(i, 0), 4):
                        nc.tensor.matmul(
                            yps_p[:, c, :],
                            ea_p[:, half * QB + c * 128 : half * QB + (c + 1) * 128],
                            v_t[tb][:, h_p, :],
                            start=False,
                            stop=(tb == 4 * b_p + c),
                            skip_group_check=True,
                        )
                if after is not None:
                    after()
                pending[0] = None

            def emit_norm(b, h, yps, ysb):
                # rec = 1/denominator (per-partition), then scale+cast
                def go():
                    rec = rpool.tile([128, 4, 1], F32, tag="rec", name=f"rec{b}_{h}")
                    nc.vector.reciprocal(rec, yps[:, :, DH : DH + 1])
                    for c in range(4):
                        nc.vector.tensor_scalar_mul(
                            ysb[:, c, h * DH : (h + 1) * DH],
                            yps[:, c, 0:DH],
                            rec[:, c, 0:1],
                        )
                return go

            for b in range(NQB):
                quanta = list(pend_prologue) + list(pend_out)
                pend_prologue = []
                pend_out = []
                if b + 1 < NQB:
                    # split pieces in half once the pair stream is long enough
                    # to absorb them one per pair
                    quanta += proj_quanta(b + 1, dmas.pop(b + 1), split=(b >= 2))
                if b + 2 < NQB:
                    dmas[b + 2] = emit_proj_dmas(b + 2)
                ysb = spool.tile([128, 4, NH3], BF, tag="ysb", name=f"ysb{b}")
                npairs = 2 * (b + 1)

                for h in range(HPC):
                    yps = None
                    for p in range(npairs):
                        sps = ps_s.tile([128, 2 * QB], F32, tag="sps", name=f"s{b}_{h}_{p}")
                        # on diag pairs put the higher tb (larger trim) in
                        # half 0 so the single exp span starts later
                        if 2 * p >= 4 * b:
                            halves = [(0, 2 * p + 1), (1, 2 * p)]
                        else:
                            halves = [(0, 2 * p), (1, 2 * p + 1)]
                        lo = 0
                        for half, tb in halves:
                            i = tb - 4 * b  # diag index if >= 0
                            ql = 128 * i if i > 0 else 0
                            if half == 0 and i >= 0:
                                lo = ql
                            off = 64 * (sc_par[0] & 1)
                            sc_par[0] += 1
                            nc.tensor.matmul(
                                sps[:, half * QB + ql : (half + 1) * QB],
                                kt_b[tb // 4][
                                    off : off + 64, h,
                                    (tb % 4) * 128 : (tb % 4 + 1) * 128,
                                ],
                                qt_b[b][off : off + 64, h, ql:QB],
                                start=True, stop=True,
                            )
                        ea = epool.tile([128, 2 * QB], BF, tag="ea")
                        nc.scalar.activation(
                            ea[:, lo : 2 * QB],
                            sps[:, lo : 2 * QB],
                            mybir.ActivationFunctionType.Exp,
                            scale=float(1.0 / np.sqrt(DH)),
                        )
                        for half, tb in halves:
                            i = tb - 4 * b
                            if i >= 0:
                                nc.vector.tensor_mul(
                                    ea[:, half * QB + 128 * i : (half + 1) * QB],
                                    ea[:, half * QB + 128 * i : (half + 1) * QB],
                                    stair[:, 0 : QB - 128 * i],
                                )
                        _flush()
                        if yps is None:
                            # allocate AFTER the previous head's norm has been
                            # emitted (by the _flush above) so the ring's
                            # reader bookkeeping is complete.
                            yps = ps_y.tile(
                                [128, 4, DH + 1], F32, tag="y", name=f"y{b}_{h}"
                            )
                            # start=True would reset has_written for the whole
                            # PSUM bank, wiping sibling chunk groups: zero the
                            # tile and accumulate with start=False throughout.
                            nc.vector.memset(yps, 0.0)
                        after = (
                            emit_norm(b, h, yps, ysb)
                            if p == npairs - 1
                            else None
                        )
                        pending[0] = (ea, halves, b, h, yps, after)
                        if quanta:
                            quanta.pop(0)()

                while quanta:
                    quanta.pop(0)()

                if b < NQB - 1:
                    pend_out = output_pieces(b, ysb)
                else:
                    _flush()
                    for q in output_pieces(b, ysb):
                        q()
    return _split_waits(nc)


_CACHE = {}


def _get_kernel():
    if "nc" not in _CACHE:
        _CACHE["nc"] = _build()
    return _CACHE["nc"]


# ---------------------------------------------------------------------------
def kernel(query, key, value, attn_mask, Wq, bq, Wk, bk, Wv, bv, Wo, bo):
    query = np.asarray(query)
    key = np.asarray(key)
    value = np.asarray(value)
    attn_mask = np.asarray(attn_mask)
    Wq, bq = np.asarray(Wq), np.asarray(bq)
    Wk, bk = np.asarray(Wk), np.asarray(bk)
    Wv, bv = np.asarray(Wv), np.asarray(bv)
    Wo, bo = np.asarray(Wo), np.asarray(bo)

    # device program hardcodes the causal staircase; verify the mask is it
    if not np.array_equal(
        attn_mask != 0, np.tril(np.ones((S, T), dtype=bool))
    ):
        raise NotImplementedError("kernel specialized for the causal mask")

    nc = _get_kernel()

    qT = [np.ascontiguousarray(query[n].T).astype(BF16) for n in range(N)]
    kT = [np.ascontiguousarray(key[n].T).astype(BF16) for n in range(N)]
    vT = [np.ascontiguousarray(value[n].T).astype(BF16) for n in range(N)]

    in_maps = []
    for core in range(8):
        n = core // 4
        g = core % 4
        cols = slice(g * NH3, (g + 1) * NH3)
        wo_aug = np.concatenate(
            [Wo[:, cols].T, (bo / 4.0)[None, :]], axis=0
        ).astype(BF16)
        in_maps.append(
            {
                "qt": qT[n],
                "kt": kT[n],
                "vt": vT[n],
                "wq": np.ascontiguousarray(Wq[cols, :].T).astype(BF16),
                "wk": np.ascontiguousarray(Wk[cols, :].T).astype(BF16),
                "wv": np.ascontiguousarray(Wv[cols, :].T).astype(BF16),
                "bqkv": np.stack(
                    [bq[cols], bk[cols], bv[cols]]
                ).astype(np.float32),
                "wo": wo_aug,
            }
        )

    trace = bool(int(os.environ.get("KERNEL_TRACE", "0")))
    res = run_bass_kernel_spmd(nc, in_maps, list(range(8)), trace=trace)
    kernel.last_exec_time_ns = res.exec_time_ns

    full = np.empty((N, S, E), np.float32)
    for n in range(N):
        acc = res.results[n * 4]["out"].astype(np.float32)
        for g in range(1, 4):
            acc = acc + res.results[n * 4 + g]["out"]
        full[n] = acc
    return full
